# revision 34
# baseline (speedup 1.0000x reference)
"""Trainium2 Bass kernel for nn_AdaptiveDeblurBlock, SPMD across 8 NeuronCores.

Host side shards batch x H-halves (4 images x 2 halves = 8 cores) with a
vertical-flip trick so one SPMD program serves both halves: every core sees
the image boundary at its top and real 4-row halo at its bottom.

V3 device program (single fully-pipelined wavefront):
  - conv1 / conv2 / conv3-softmax / tap-reduce / f1 / f2 all interleave per
    4-row superblock so the PE never drains and DVE work hides under PE.
  - softmax = exp(l - 4) -> 9-partition sum via ones-matmul -> one GPSIMD
    divide K = E/S (no ln/exp recompute, no act-table thrash: every
    activation func used lives in one act-func set).
  - tap-reduce reads the resident x slab directly (no DRAM re-fetch).
  - 4-row conv blocks: two 2-row matmul groups into one 2-bank PSUM tile,
    drained by a single strided activation (halves ACT instruction count).
"""
import os, sys
for _p in ("/opt/trn_rl_repo",):
    if _p not in sys.path and os.path.isdir(_p):
        sys.path.append(_p)

from contextlib import ExitStack

import concourse.bass as bass
import concourse.bacc as bacc
import concourse.tile as tile
import concourse.mybir as mybir

DT16 = mybir.dt.float16
F32 = mybir.dt.float32
AF = mybir.ActivationFunctionType
ALU = mybir.AluOpType


def build_kernel(RIN=100, W=192, C=128, RB=4, compile_=True, with_tick=True,
                 div_on_pool=True):
    Wp = W + 2
    nX = RIN + 1          # XB rows -1..RIN-1
    nC1 = RIN             # C1 rows -1..RIN-2 (computed 0..RIN-2)
    nC2 = RIN - 2         # C2 rows 0..RIN-3
    nA = RIN - 1          # A rows -1..RIN-3
    HOUT = RIN - 4

    rows_c1 = RIN - 1     # 99
    rows_c2 = RIN - 2     # 98
    rows_k = RIN - 2      # 98
    rows_f1 = RIN - 3     # 97
    rows_f2 = RIN - 4     # 96

    def flatsz(nrows):
        return 1 + nrows * Wp + 1

    nc = bacc.Bacc()
    xb_ext = nc.declare_dram_parameter("xb", [C, flatsz(nX)], DT16, isOutput=False)
    w1_ext = nc.declare_dram_parameter("w1l", [128, 9 * 64], DT16, isOutput=False)
    w2_ext = nc.declare_dram_parameter("w2l", [128, 6 * 32], DT16, isOutput=False)
    w3_ext = nc.declare_dram_parameter("w3l", [32, 9], DT16, isOutput=False)
    f1_ext = nc.declare_dram_parameter("f1l", [128, 9 * 128], DT16, isOutput=False)
    f2_ext = nc.declare_dram_parameter("f2l", [128, 9 * 128], DT16, isOutput=False)
    on_ext = nc.declare_dram_parameter("ones", [16, 16], DT16, isOutput=False)
    bi_ext = nc.declare_dram_parameter("bias", [128, 8], F32, isOutput=False)
    tick_ext = tock_ext = None
    if with_tick:
        tick_ext = nc.declare_dram_parameter("tick", [1, 16], F32, isOutput=False)
    out_ext = nc.declare_dram_parameter("out", [C, HOUT * W], F32, isOutput=True)
    if with_tick:
        tock_ext = nc.declare_dram_parameter("tock", [1, 16], F32, isOutput=True)

    kdram = nc.dram_tensor("kdram", [9, rows_k * Wp + 2], DT16)

    def rview(t, off, nr, c0, c1):
        return t[:, off:off + nr * Wp].rearrange("p (r w) -> p r w", w=Wp)[:, :, c0:c1]

    with tile.TileContext(nc) as tc, ExitStack() as ctx:
        wpool = ctx.enter_context(tc.tile_pool(name="wpool", bufs=1))
        big = ctx.enter_context(tc.tile_pool(name="big", bufs=1))
        mid = ctx.enter_context(tc.tile_pool(name="mid", bufs=2))
        ksp = ctx.enter_context(tc.tile_pool(name="ksp", bufs=2))
        kbp = ctx.enter_context(tc.tile_pool(name="kbp", bufs=2))
        stg = ctx.enter_context(tc.tile_pool(name="stg", bufs=2))
        psum = ctx.enter_context(tc.tile_pool(name="psum", bufs=2, space="PSUM"))
        psum3 = ctx.enter_context(tc.tile_pool(name="psum3", bufs=2, space="PSUM"))
        psums = ctx.enter_context(tc.tile_pool(name="psums", bufs=2, space="PSUM"))

        if with_tick:
            nc.sync.dma_start(out=tock_ext[:], in_=tick_ext[:])

        W1 = wpool.tile([128, 9 * 64], DT16, tag="w1")
        W2 = wpool.tile([128, 6 * 32], DT16, tag="w2")
        W3 = wpool.tile([32, 9], DT16, tag="w3")
        F1W = wpool.tile([128, 9 * 128], DT16, tag="f1")
        F2W = wpool.tile([128, 9 * 128], DT16, tag="f2")
        ONES = wpool.tile([16, 16], DT16, tag="ones")
        BIAS = wpool.tile([128, 8], F32, tag="bias")
        # conv1's weights/bias first; the f1/f2 weights aren't needed for
        # tens of microseconds, so they queue behind the first x chunks
        nc.sync.dma_start(out=W1[:], in_=w1_ext[:])
        nc.sync.dma_start(out=BIAS[:], in_=bi_ext[:])
        b1 = BIAS[0:64, 0:1]
        b2 = BIAS[0:32, 1:2]
        b3 = BIAS[0:9, 2:3]
        fb1 = BIAS[0:128, 3:4]
        fb2 = BIAS[0:128, 4:5]
        ones9 = ONES[0:9, 0:9]      # lhsT for the 9-partition softmax sum

        # x slab, chunked so conv1's first blocks start early; stays resident
        # for the tap-reduce (taps read it directly, any alignment).
        XB = big.tile([C, flatsz(nX)], DT16, tag="xb")
        pos = 0
        chunks = iter([8, 8, 16, 24])
        for rows in list([8, 8, 16]):
            end = min(pos + rows * Wp, flatsz(nX))
            nc.sync.dma_start(out=XB[:, pos:end], in_=xb_ext[:, pos:end])
            pos = end
        nc.sync.dma_start(out=W2[:], in_=w2_ext[:])
        nc.sync.dma_start(out=W3[:], in_=w3_ext[:])
        nc.sync.dma_start(out=ONES[:], in_=on_ext[:])
        while pos < flatsz(nX):
            end = min(pos + 24 * Wp, flatsz(nX))
            nc.sync.dma_start(out=XB[:, pos:end], in_=xb_ext[:, pos:end])
            pos = end
        nc.sync.dma_start(out=F1W[:], in_=f1_ext[:])
        nc.sync.dma_start(out=F2W[:], in_=f2_ext[:])

        def pad_memset(buf, nrows, zero_top_row):
            if zero_top_row:
                nc.vector.memset(buf[:, 0:1 + Wp + 1], 0)
                first_pair_row = 1
            else:
                nc.vector.memset(buf[:, 0:2], 0)
                first_pair_row = 0
            npair = (nrows - 1) - first_pair_row
            if npair > 0:
                s = 1 + first_pair_row * Wp + (Wp - 1)
                nc.vector.memset(
                    buf[:, s:s + npair * Wp].rearrange(
                        "p (r w) -> p r w", w=Wp)[:, :, 0:2], 0)
            s = 1 + (nrows - 1) * Wp + (Wp - 1)
            nc.vector.memset(buf[:, s:s + 2], 0)

        C1 = big.tile([128, flatsz(nC1)], DT16, tag="slab1")
        C2 = big.tile([32, flatsz(nC2)], DT16, tag="slab2")
        # A (reduce output) and F1 share one slab: F1 row r sits at A row
        # r-2's position, which every f1 consumer has already read by the
        # time f1's activation writes it (f1 trails the reduce frontier by
        # >= 2 rows, and future f1 blocks only read A rows >= r+3).
        SAF = big.tile([C, 2 * Wp + flatsz(nA)], DT16, tag="slabAF")
        A = SAF[:, 2 * Wp:2 * Wp + flatsz(nA)]
        F1 = SAF[:, 0:flatsz(nC1 - 2)]
        pad_memset(C1, nC1, True)
        pad_memset(C2, nC2, False)
        pad_memset(A, nA, True)
        nc.vector.memset(SAF[:, 0:1 + Wp + 1], 0)   # F1's -1 halo row

        # ---- conv blocks: up to 4 rows as two 2-row PSUM groups ----
        def conv_mm(r, nr, wtile, m, k, src, PT, base):
            N = nr * Wp
            for t in range(9):
                di, dj = divmod(t, 3)
                off = 1 + (r + di) * Wp + (dj - 1)
                nc.tensor.matmul(PT[:m, base:base + N], wtile[:k, t * m:(t + 1) * m],
                                 src[:k, off:off + N], start=(t == 0), stop=(t == 8))

        def act_drain(PT, m, halves, dst, doff, bias_ap, func):
            # halves: list of (base, nr); dst row offset doff (flat elem index
            # of (row, col0)); writes via one ACT per half unless 2+2 merged.
            if len(halves) == 2 and halves[0][1] == 2 and halves[1][1] == 2:
                pin = PT[:m, 0:1024].rearrange("p (b n) -> p b n", n=512)[:, :, 0:2 * Wp] \
                    .rearrange("p b (r w) -> p b r w", w=Wp)[:, :, :, 1:1 + W]
                pout = dst[:, doff:doff + 4 * Wp].rearrange(
                    "p (b r w) -> p b r w", b=2, w=Wp)[:, :, :, 0:W]
                nc.scalar.activation(pout, pin, func, bias=bias_ap)
            else:
                row = 0
                for base, nr in halves:
                    N = nr * Wp
                    psrc = PT[:m, base:base + N].rearrange(
                        "p (r w) -> p r w", w=Wp)[:, :, 1:1 + W]
                    pdst = rview(dst, doff + row * Wp, nr, 0, W)
                    nc.scalar.activation(pdst, psrc, func, bias=bias_ap)
                    row += nr

        def conv_block4(r, nr, wtile, m, k, src, bias_ap, dst, func):
            # output rows r..r+nr-1 (nr<=4), dst has a -1 halo row
            PT = psum.tile([128, 1024], F32, tag="mm")
            halves = []
            rh, left = r, nr
            base = 0
            while left > 0:
                nh = min(2, left)
                conv_mm(rh, nh, wtile, m, k, src, PT, base)
                halves.append((base, nh))
                rh += nh
                left -= nh
                base += 512
            act_drain(PT, m, halves, dst, 1 + (r + 1) * Wp + 1, bias_ap, func)

        def c2_mm(r, nr, PT, base):
            N = nr * Wp
            for dj in range(3):
                off = 1 + r * Wp + dj - 1                 # di=0 base (pair di=0,1)
                nc.tensor.matmul(PT[:32, base:base + N], W2[0:128, dj * 32:(dj + 1) * 32],
                                 C1[0:128, off:off + N],
                                 start=(dj == 0), stop=False)
            for dj in range(3):
                off = 1 + (r + 2) * Wp + dj - 1           # di=2
                nc.tensor.matmul(PT[:32, base:base + N], W2[0:64, (3 + dj) * 32:(4 + dj) * 32],
                                 C1[0:64, off:off + N],
                                 start=False, stop=(dj == 2))

        def c2_block4(r, nr):
            PT = psum.tile([128, 1024], F32, tag="mm")
            halves = []
            rh, left, base = r, nr, 0
            while left > 0:
                nh = min(2, left)
                c2_mm(rh, nh, PT, base)
                halves.append((base, nh))
                rh += nh
                left -= nh
                base += 512
            act_drain(PT, 32, halves, C2, 1 + r * Wp + 1, b2, AF.Relu)

        def c1_block4(i):
            r = 4 * i
            nr = min(4, rows_c1 - r)
            conv_block4(r, nr, W1, 64, 128, XB, b1, C1[0:64, :], AF.Relu)
            # one-row-shifted partition copy for conv2's K-packed tap pairs;
            # triggered from the ACT queue so it never head-of-line-blocks
            # the SP queue (which carries the latency-critical kdram writes)
            s0 = 1 + (r + 1) * Wp
            ncopy = min(nr * Wp, flatsz(nC1) - 1 - s0)
            nc.scalar.dma_start(out=C1[64:128, s0 - Wp:s0 - Wp + ncopy],
                                in_=C1[0:64, s0:s0 + ncopy])

        # ---- conv3 + softmax: K = exp(l-4) / sum ----
        # split in two phases so the PE never waits on the E activation:
        # logits+exp are emitted first, conv blocks fill the PE while the
        # ACT runs, then the sum-matmul finds E already in SBUF.
        def c3_logits_exp(r, nr):
            N = nr * Wp
            co = 1 + r * Wp
            lg = psum3.tile([9, 512], F32, tag="c3a")
            nc.tensor.matmul(lg[:, :N], W3[:, :], C2[:, co:co + N],
                             start=True, stop=True)
            E = mid.tile([9, 512], DT16, tag="E")
            nc.scalar.activation(E[:, :N], lg[:, :N], AF.Exp, bias=b3)
            return E

        def c3_sum_div(r, nr, E, KS, kcol):
            N = nr * Wp
            sp9 = psums.tile([9, 512], F32, tag="s")
            nc.tensor.matmul(sp9[:, :N], ones9, E[:, :N], start=True, stop=True)
            # DVE has no divide ALU: native reciprocal (straight from PSUM,
            # replicated to all 9 partitions by the ones-matmul) + multiply
            R9 = mid.tile([9, 512], DT16, tag="R")
            with nc.allow_low_precision(reason="1/S in fp16: S in [1e-2, 5e2]"):
                nc.vector.reciprocal(R9[:, :N], sp9[:, :N])
            nc.vector.tensor_mul(KS[:, kcol:kcol + N], E[:, :N], R9[:, :N])

        # ---- tap reduce: A[c,f] = sum_t K[t,f] * x[c, f+delta_t] ----
        def bcast_block(r0, nr):
            F = nr * Wp
            KB = kbp.tile([C, 2 + 9 * RB * Wp], DT16, tag="kb")
            src = kdram[0:9, r0 * Wp:r0 * Wp + F].partition_broadcast(C)
            nc.gpsimd.dma_start(
                out=KB[:, 1:1 + 9 * F].rearrange("p (t f) -> p t f", f=F), in_=src)
            return KB

        def reduce_block(r0, nr, KB):
            # 7 DVE ops instead of 17: each op handles 3 tap-planes at once
            # (x taps via an overlapping stride-1 tap dim; products written
            # in-place over KB's k planes), then a 2-level fold into A.
            F = nr * Wp
            xps = list(XB.ap)[0][0]
            kps = list(KB.ap)[0][0]

            def x3(di):
                return bass.AP(XB.tensor, XB.offset + 1 + (r0 + di) * Wp,
                               [[xps, C], [1, 3], [Wp, nr], [1, W]])

            def k3(di):
                return bass.AP(KB.tensor, KB.offset + 1 + (3 * di) * F + 1,
                               [[kps, C], [F, 3], [Wp, nr], [1, W]])

            def k1(t):
                return bass.AP(KB.tensor, KB.offset + 1 + t * F + 1,
                               [[kps, C], [Wp, nr], [1, W]])

            for di in range(3):
                nc.vector.tensor_mul(k3(di), x3(di), k3(di))
            nc.vector.tensor_add(k3(0), k3(0), k3(1))
            nc.vector.tensor_add(k3(0), k3(0), k3(2))
            av = rview(A, 1 + (r0 + 1) * Wp + 1, nr, 0, W)
            nc.vector.tensor_add(av, k1(0), k1(1))
            nc.vector.tensor_add(av, av, k1(2))

        def f2_block(r, nr):
            PT = psum.tile([128, 1024], F32, tag="mm")
            halves = []
            rh, left, base = r, nr, 0
            while left > 0:
                nh = min(2, left)
                N = nh * Wp
                for t in range(9):
                    di, dj = divmod(t, 3)
                    off = 1 + (rh + di) * Wp + (dj - 1)
                    nc.tensor.matmul(PT[:, base:base + N], F2W[:, t * 128:(t + 1) * 128],
                                     F1[:, off:off + N], start=(t == 0), stop=(t == 8))
                halves.append((base, nh))
                rh += nh
                left -= nh
                base += 512
            ST = stg.tile([128, 4 * W], F32, tag="st")
            if len(halves) == 2 and halves[1][1] == 2:
                pin = PT[:, 0:1024].rearrange("p (b n) -> p b n", n=512)[:, :, 0:2 * Wp] \
                    .rearrange("p b (r w) -> p b r w", w=Wp)[:, :, :, 1:1 + W]
                pout = ST[:, 0:4 * W].rearrange("p (b r w) -> p b r w", b=2, w=W)
                nc.scalar.activation(pout, pin, AF.Identity, bias=fb2)
            else:
                row = 0
                for base, nh in halves:
                    N = nh * Wp
                    psrc = PT[:, base:base + N].rearrange(
                        "p (r w) -> p r w", w=Wp)[:, :, 1:1 + W]
                    pdst = ST[:, row * W:(row + nh) * W].rearrange(
                        "p (r w) -> p r w", w=W)
                    nc.scalar.activation(pdst, psrc, AF.Identity, bias=fb2)
                    row += nh
            nc.sync.dma_start(out=out_ext[:, r * W:(r + nr) * W], in_=ST[:, :nr * W])

        blocksf1 = [(r, min(4, rows_f1 - r)) for r in range(0, rows_f1, 4)]
        blocksf2 = [(r, min(4, rows_f2 - r)) for r in range(0, rows_f2, 4)]
        state = {"if1": 0, "if2": 0, "f1_rows": 0}

        def emit_one_f(a_rows, final=False):
            if state["if2"] < len(blocksf2):
                r2, n2 = blocksf2[state["if2"]]
                if final or r2 + n2 <= state["f1_rows"] - 1:
                    f2_block(r2, n2)
                    state["if2"] += 1
                    return True
            if state["if1"] < len(blocksf1):
                r, nr = blocksf1[state["if1"]]
                if final or r + nr <= a_rows - 1:
                    conv_block4(r, nr, F1W, 128, 128, A, fb1, F1, AF.Relu)
                    state["if1"] += 1
                    state["f1_rows"] = r + nr
                    return True
            return False

        # ---- fully pipelined emission ----
        n_c1 = (rows_c1 + 3) // 4      # 25
        n_c2 = (rows_c2 + 3) // 4      # 25
        i_c1 = i_c2 = 0
        sub_r = 0

        def c2_step(j):
            r = 4 * j
            c2_block4(r, min(4, rows_c2 - r))

        # prologue: build a 6-block c1 / 4-block c2 lead so the PE stays fed
        # while the first K-broadcast chains fill the pipeline
        for _ in range(6):
            c1_block4(i_c1); i_c1 += 1
        for _ in range(4):
            c2_step(i_c2); i_c2 += 1

        sblocks = [(r0, min(RB, rows_k - r0)) for r0 in range(0, rows_k, RB)]
        n_sb = len(sblocks)
        cstate = {"c1": i_c1, "c2": i_c2}
        kb_q = []

        def filler():
            if cstate["c1"] < n_c1:
                c1_block4(cstate["c1"]); cstate["c1"] += 1
            if cstate["c2"] < n_c2:
                c2_step(cstate["c2"]); cstate["c2"] += 1

        def k_front(ik, pe_filler=None):
            # c3 chain + kdram + broadcast for superblock ik, emitted one
            # superblock ahead of its reduce so the DVE divides precede the
            # previous reduce's TT batch and the broadcast overlaps it
            r0, nr = sblocks[ik]
            KS = ksp.tile([9, RB * Wp + 2], DT16, tag="ks")
            subs = []
            rr = r0
            while rr < r0 + nr:
                n2 = min(2, rows_k - rr)
                subs.append((rr, n2, c3_logits_exp(rr, n2)))
                rr += n2
            if pe_filler:
                pe_filler()          # PE work while the exp activations drain
            for (rr, n2, E) in subs:
                c3_sum_div(rr, n2, E, KS, (rr - r0) * Wp)
            nc.sync.dma_start(out=kdram[0:9, r0 * Wp:r0 * Wp + nr * Wp],
                              in_=KS[0:9, 0:nr * Wp])
            kb_q.append((r0, nr, bcast_block(r0, nr)))

        k_front(0)
        k_rows = sblocks[0][1]
        for s in range(n_sb):
            if s + 1 < n_sb:
                k_front(s + 1, filler)
                k_rows += sblocks[s + 1][1]
            else:
                filler()
            reduce_block(*kb_q.pop(0))
            # f blocks trail the reduce cadence by ~2 superblocks so the PE
            # (in-order) never parks on an unready f block
            a_gate = k_rows - sblocks[s][1] - 2 * RB
            emit_one_f(a_gate)
            emit_one_f(a_gate)
        # drain: emit_one_f's f2-first gating yields the stall-minimal
        # interleaving of the remaining blocks
        while emit_one_f(rows_k):
            pass
        while state["if1"] < len(blocksf1) or state["if2"] < len(blocksf2):
            emit_one_f(rows_k, final=True)

    if compile_:
        nc.compile()
    return nc


# ==========================================================================
# host side
# ==========================================================================
import numpy as np
from concourse.bass_utils import run_bass_kernel_spmd

NP16 = np.float16
B, C, H, W = 4, 128, 192, 192
HB = H // 2
RIN = HB + 4
Wp = W + 2

_FLIP_PERM = np.array([3 * (2 - (t // 3)) + (t % 3) for t in range(9)])

LOGIT_SHIFT = 4.0   # exp(l - 4): safe for logits in (-12.5, 12.9) per-pixel max


def _flip_taps(w):
    return np.ascontiguousarray(w[:, :, ::-1, :])


def _pack_weights(w1, b1, w2, b2, w3, b3, f1, fb1, f2, fb2, flipped):
    if flipped:
        w1, w2, f1, f2 = _flip_taps(w1), _flip_taps(w2), _flip_taps(f1), _flip_taps(f2)
        w3 = w3[_FLIP_PERM]
        b3 = b3[_FLIP_PERM]
    d = {}
    d["w1l"] = np.ascontiguousarray(
        w1.transpose(1, 2, 3, 0).reshape(C, 9 * 64)).astype(NP16)
    # conv2 K-packed lhsT [128, 6*32]: cols dj*32 hold the di=0/1 pair
    # (rows 0-63 di=0, rows 64-127 di=1); cols (3+dj)*32 hold di=2 (rows 0-63)
    w2p = np.zeros((128, 6 * 32), np.float32)
    for dj in range(3):
        w2p[0:64, dj * 32:(dj + 1) * 32] = w2[:, :, 0, dj].T
        w2p[64:128, dj * 32:(dj + 1) * 32] = w2[:, :, 1, dj].T
        w2p[0:64, (3 + dj) * 32:(4 + dj) * 32] = w2[:, :, 2, dj].T
    d["w2l"] = w2p.astype(NP16)
    d["w3l"] = np.ascontiguousarray(w3[:, :, 0, 0].T).astype(NP16)
    d["f1l"] = np.ascontiguousarray(
        f1.transpose(1, 2, 3, 0).reshape(C, 9 * C)).astype(NP16)
    d["f2l"] = np.ascontiguousarray(
        f2.transpose(1, 2, 3, 0).reshape(C, 9 * C)).astype(NP16)
    ones = np.zeros((16, 16), np.float32)
    ones[0:9, 0:9] = 1.0
    d["ones"] = ones.astype(NP16)
    bias = np.zeros((C, 8), np.float32)
    bias[0:64, 0] = b1
    bias[0:32, 1] = b2
    # logit shift keeps exp() within fp16 range; cancels exactly in K = E/S
    bias[0:9, 2] = b3 - LOGIT_SHIFT
    bias[0:C, 3] = fb1
    bias[0:C, 4] = fb2
    d["bias"] = bias
    d["tick"] = np.zeros((1, 16), np.float32)
    return d


def _pack_x(x_loc):
    nX = RIN + 1
    xb = np.zeros((C, 1 + nX * Wp + 1), np.float32)
    v = xb[:, 1:1 + nX * Wp].reshape(C, nX, Wp)
    v[:, 1:, 1:1 + W] = x_loc
    return xb.astype(NP16)


_NC_CACHE = {}


def _get_nc():
    if "nc" not in _NC_CACHE:
        _NC_CACHE["nc"] = build_kernel()
    return _NC_CACHE["nc"]


def _make_in_maps(x, w1, b1, w2, b2, w3, b3, f1, fb1, f2, fb2):
    x = np.asarray(x, np.float32)
    args = [np.asarray(a, np.float32) for a in
            (w1, b1, w2, b2, w3, b3, f1, fb1, f2, fb2)]
    wtop = _pack_weights(*args, flipped=False)
    wbot = _pack_weights(*args, flipped=True)
    in_maps = []
    for b in range(B):
        for j in range(2):
            if j == 0:
                xloc = x[b, :, 0:RIN, :]
                wd = wtop
            else:
                xloc = np.ascontiguousarray(x[b, :, H - RIN:H, :][:, ::-1, :])
                wd = wbot
            m = dict(wd)
            m["xb"] = _pack_x(xloc)
            in_maps.append(m)
    return in_maps


def _assemble(results):
    out = np.zeros((B, C, H, W), np.float32)
    for b in range(B):
        out[b, :, 0:HB, :] = results[2 * b]["out"].reshape(C, HB, W)
        out[b, :, HB:H, :] = results[2 * b + 1]["out"].reshape(C, HB, W)[:, ::-1, :]
    return out


def kernel(x, w1, b1, w2, b2, w3, b3, f1, fb1, f2, fb2):
    nc = _get_nc()
    in_maps = _make_in_maps(x, w1, b1, w2, b2, w3, b3, f1, fb1, f2, fb2)
    res = run_bass_kernel_spmd(nc, in_maps, core_ids=list(range(8)))
    return _assemble(res.results)


# revision 71
# speedup vs baseline: 1.1442x; 1.1442x over previous
"""Trainium2 Bass kernel for nn_AdaptiveDeblurBlock, SPMD across 8 NeuronCores.

Host side shards batch x H-halves (4 images x 2 halves = 8 cores) with a
vertical-flip trick so one SPMD program serves both halves: every core sees
the image boundary at its top and real 4-row halo at its bottom.

V3 device program (single fully-pipelined wavefront):
  - conv1 / conv2 / conv3-softmax / tap-reduce / f1 / f2 all interleave per
    4-row superblock so the PE never drains and DVE work hides under PE.
  - softmax = exp(l - 4) -> 9-partition sum via ones-matmul -> one GPSIMD
    divide K = E/S (no ln/exp recompute, no act-table thrash: every
    activation func used lives in one act-func set).
  - tap-reduce reads the resident x slab directly (no DRAM re-fetch).
  - 4-row conv blocks: two 2-row matmul groups into one 2-bank PSUM tile,
    drained by a single strided activation (halves ACT instruction count).
"""
import os, sys
for _p in ("/opt/trn_rl_repo",):
    if _p not in sys.path and os.path.isdir(_p):
        sys.path.append(_p)

from contextlib import ExitStack

import concourse.bass as bass
import concourse.bacc as bacc
import concourse.tile as tile
import concourse.mybir as mybir

DT16 = mybir.dt.float16
F32 = mybir.dt.float32
AF = mybir.ActivationFunctionType
ALU = mybir.AluOpType


def build_kernel(RIN=100, W=192, C=128, RB=4, compile_=True, with_tick=False,
                 div_on_pool=True):
    Wp = W + 2
    nX = RIN + 1          # XB rows -1..RIN-1
    nC1 = RIN             # C1 rows -1..RIN-2 (computed 0..RIN-2)
    nC2 = RIN - 2         # C2 rows 0..RIN-3
    nA = RIN - 1          # A rows -1..RIN-3
    HOUT = RIN - 4

    rows_c1 = RIN - 1     # 99
    rows_c2 = RIN - 2     # 98
    rows_k = RIN - 2      # 98
    rows_f1 = RIN - 3     # 97
    rows_f2 = RIN - 4     # 96

    def flatsz(nrows):
        return 1 + nrows * Wp + 1

    nc = bacc.Bacc()
    xb_ext = nc.declare_dram_parameter("xb", [C, flatsz(nX)], DT16, isOutput=False)
    w1_ext = nc.declare_dram_parameter("w1l", [128, 9 * 64], DT16, isOutput=False)
    w2_ext = nc.declare_dram_parameter("w2l", [128, 6 * 32], DT16, isOutput=False)
    w3_ext = nc.declare_dram_parameter("w3l", [32, 9], DT16, isOutput=False)
    f1_ext = nc.declare_dram_parameter("f1l", [128, 9 * 128], DT16, isOutput=False)
    f2_ext = nc.declare_dram_parameter("f2l", [128, 9 * 128], DT16, isOutput=False)
    on_ext = nc.declare_dram_parameter("ones", [16, 16], DT16, isOutput=False)
    bi_ext = nc.declare_dram_parameter("bias", [128, 8], F32, isOutput=False)
    tick_ext = tock_ext = None
    if with_tick:
        tick_ext = nc.declare_dram_parameter("tick", [1, 16], F32, isOutput=False)
    out_ext = nc.declare_dram_parameter("out", [C, HOUT * W], F32, isOutput=True)
    if with_tick:
        tock_ext = nc.declare_dram_parameter("tock", [1, 16], F32, isOutput=True)

    kdram = nc.dram_tensor("kdram", [9, rows_k * Wp + 2], DT16)

    def rview(t, off, nr, c0, c1):
        return t[:, off:off + nr * Wp].rearrange("p (r w) -> p r w", w=Wp)[:, :, c0:c1]

    with tile.TileContext(nc) as tc, ExitStack() as ctx:
        wpool = ctx.enter_context(tc.tile_pool(name="wpool", bufs=1))
        big = ctx.enter_context(tc.tile_pool(name="big", bufs=1))
        mid = ctx.enter_context(tc.tile_pool(name="mid", bufs=2))
        ksp = ctx.enter_context(tc.tile_pool(name="ksp", bufs=2))
        kbp = ctx.enter_context(tc.tile_pool(name="kbp", bufs=2))
        stg = ctx.enter_context(tc.tile_pool(name="stg", bufs=2))
        psum = ctx.enter_context(tc.tile_pool(name="psum", bufs=2, space="PSUM"))
        psum3 = ctx.enter_context(tc.tile_pool(name="psum3", bufs=2, space="PSUM"))
        psums = ctx.enter_context(tc.tile_pool(name="psums", bufs=2, space="PSUM"))

        if with_tick:
            nc.sync.dma_start(out=tock_ext[:], in_=tick_ext[:])

        W1 = wpool.tile([128, 9 * 64], DT16, tag="w1")
        W2 = wpool.tile([128, 6 * 32], DT16, tag="w2")
        W3 = wpool.tile([32, 9], DT16, tag="w3")
        F1W = wpool.tile([128, 9 * 128], DT16, tag="f1")
        F2W = wpool.tile([128, 9 * 128], DT16, tag="f2")
        ONES = wpool.tile([16, 16], DT16, tag="ones")
        BIAS = wpool.tile([128, 8], F32, tag="bias")
        # conv1's first-taps weights land first (the opening ldweights only
        # needs tap 0); the rest follow the first x chunk
        nc.sync.dma_start(out=W1[:, 0:192], in_=w1_ext[:, 0:192])
        b1 = BIAS[0:64, 0:1]
        b2 = BIAS[0:32, 1:2]
        b3 = BIAS[0:9, 2:3]
        fb1 = BIAS[0:128, 3:4]
        fb2 = BIAS[0:128, 4:5]
        ones9 = ONES[0:9, 0:9]      # lhsT for the 9-partition softmax sum

        # x slab, chunked so conv1's first blocks start early; stays resident
        # for the tap-reduce (taps read it directly, any alignment).
        XB = big.tile([C, flatsz(nX)], DT16, tag="xb")
        # Only the first ~31 x rows load up front; the rest stagger into the
        # loop iterations so bulk transfers never monopolize the DMA engines
        # ahead of the latency-critical K-broadcast chain. Same for the f1/f2
        # weights (first needed tens of microseconds in).
        xstate = {"pos": 0}

        def emit_xchunk(rows):
            if xstate["pos"] >= flatsz(nX):
                return
            end = min(xstate["pos"] + rows * Wp, flatsz(nX))
            nc.sync.dma_start(out=XB[:, xstate["pos"]:end],
                              in_=xb_ext[:, xstate["pos"]:end])
            xstate["pos"] = end

        emit_xchunk(6)      # rows -1..4: exactly c1 block 0's window
        nc.sync.dma_start(out=W1[:, 192:576], in_=w1_ext[:, 192:576])
        emit_xchunk(4)
        nc.sync.dma_start(out=BIAS[:], in_=bi_ext[:])
        emit_xchunk(6)
        emit_xchunk(16)
        nc.sync.dma_start(out=W2[:], in_=w2_ext[:])
        nc.sync.dma_start(out=W3[:], in_=w3_ext[:])
        nc.sync.dma_start(out=ONES[:], in_=on_ext[:])
        # coverage invariant: every XB row a conv/tap reads must have its
        # chunk EMITTED before the reader, else tile inserts no dependency
        # and hardware reads uninitialized SBUF. Prologue covers rows -1..46;
        # the loop emits 16 rows/iter ahead of each k_front (reader horizon
        # grows 4 rows/iter).
        emit_xchunk(16)

        def pad_memset(buf, nrows, zero_top_row):
            # on GPSIMD so the early DVE queue stays clear for the reduce
            if zero_top_row:
                nc.gpsimd.memset(buf[:, 0:1 + Wp + 1], 0)
                first_pair_row = 1
            else:
                nc.gpsimd.memset(buf[:, 0:2], 0)
                first_pair_row = 0
            npair = (nrows - 1) - first_pair_row
            if npair > 0:
                s = 1 + first_pair_row * Wp + (Wp - 1)
                nc.gpsimd.memset(
                    buf[:, s:s + npair * Wp].rearrange(
                        "p (r w) -> p r w", w=Wp)[:, :, 0:2], 0)
            s = 1 + (nrows - 1) * Wp + (Wp - 1)
            nc.gpsimd.memset(buf[:, s:s + 2], 0)

        C1 = big.tile([128, flatsz(nC1)], DT16, tag="slab1")
        C2 = big.tile([32, flatsz(nC2)], DT16, tag="slab2")
        # A (reduce output) and F1 share one slab: F1 row r sits at A row
        # r-2's position, which every f1 consumer has already read by the
        # time f1's activation writes it (f1 trails the reduce frontier by
        # >= 2 rows, and future f1 blocks only read A rows >= r+3).
        SAF = big.tile([C, 2 * Wp + flatsz(nA)], DT16, tag="slabAF")
        A = SAF[:, 2 * Wp:2 * Wp + flatsz(nA)]
        F1 = SAF[:, 0:flatsz(nC1 - 2)]
        pad_memset(C1, nC1, True)
        pad_memset(C2, nC2, False)
        pad_memset(A, nA, True)
        nc.gpsimd.memset(SAF[:, 0:1 + Wp + 1], 0)   # F1's -1 halo row

        # ---- conv blocks: up to 4 rows as two 2-row PSUM groups ----
        def conv_mm(r, nr, wtile, m, k, src, PT, base):
            N = nr * Wp
            for t in range(9):
                di, dj = divmod(t, 3)
                off = 1 + (r + di) * Wp + (dj - 1)
                nc.tensor.matmul(PT[:m, base:base + N], wtile[:k, t * m:(t + 1) * m],
                                 src[:k, off:off + N], start=(t == 0), stop=(t == 8))

        def act_drain(PT, m, halves, dst, doff, bias_ap, func):
            # halves: list of (base, nr); dst row offset doff (flat elem index
            # of (row, col0)); writes via one ACT per half unless 2+2 merged.
            if len(halves) == 2 and halves[0][1] == 2 and halves[1][1] == 2:
                pin = PT[:m, 0:1024].rearrange("p (b n) -> p b n", n=512)[:, :, 0:2 * Wp] \
                    .rearrange("p b (r w) -> p b r w", w=Wp)[:, :, :, 1:1 + W]
                pout = dst[:, doff:doff + 4 * Wp].rearrange(
                    "p (b r w) -> p b r w", b=2, w=Wp)[:, :, :, 0:W]
                nc.scalar.activation(pout, pin, func, bias=bias_ap)
            else:
                row = 0
                for base, nr in halves:
                    N = nr * Wp
                    psrc = PT[:m, base:base + N].rearrange(
                        "p (r w) -> p r w", w=Wp)[:, :, 1:1 + W]
                    pdst = rview(dst, doff + row * Wp, nr, 0, W)
                    nc.scalar.activation(pdst, psrc, func, bias=bias_ap)
                    row += nr

        def conv_block4(r, nr, wtile, m, k, src, bias_ap, dst, func):
            # output rows r..r+nr-1 (nr<=4), dst has a -1 halo row
            PT = psum.tile([128, 1024], F32, tag="mm")
            halves = []
            rh, left = r, nr
            base = 0
            while left > 0:
                nh = min(2, left)
                conv_mm(rh, nh, wtile, m, k, src, PT, base)
                halves.append((base, nh))
                rh += nh
                left -= nh
                base += 512
            act_drain(PT, m, halves, dst, 1 + (r + 1) * Wp + 1, bias_ap, func)

        def c2_mm(r, nr, PT, base):
            N = nr * Wp
            for dj in range(3):
                off = 1 + r * Wp + dj - 1                 # di=0 base (pair di=0,1)
                nc.tensor.matmul(PT[:32, base:base + N], W2[0:128, dj * 32:(dj + 1) * 32],
                                 C1[0:128, off:off + N],
                                 start=(dj == 0), stop=False)
            for dj in range(3):
                off = 1 + (r + 2) * Wp + dj - 1           # di=2
                nc.tensor.matmul(PT[:32, base:base + N], W2[0:64, (3 + dj) * 32:(4 + dj) * 32],
                                 C1[0:64, off:off + N],
                                 start=False, stop=(dj == 2))

        def c2_block4(r, nr):
            PT = psum.tile([128, 1024], F32, tag="mm")
            halves = []
            rh, left, base = r, nr, 0
            while left > 0:
                nh = min(2, left)
                c2_mm(rh, nh, PT, base)
                halves.append((base, nh))
                rh += nh
                left -= nh
                base += 512
            act_drain(PT, 32, halves, C2, 1 + r * Wp + 1, b2, AF.Relu)

        def c1_block4(i):
            r = 4 * i
            nr = min(4, rows_c1 - r)
            conv_block4(r, nr, W1, 64, 128, XB, b1, C1[0:64, :], AF.Relu)
            # one-row-shifted partition copy for conv2's K-packed tap pairs;
            # triggered from the ACT queue (its wait is satisfied the moment
            # ACT.SEQ reaches it: the producing activation just ran); SP
            # carries the latency-critical kdram writes
            s0 = 1 + (r + 1) * Wp
            ncopy = min(nr * Wp, flatsz(nC1) - 1 - s0)
            nc.scalar.dma_start(out=C1[64:128, s0 - Wp:s0 - Wp + ncopy],
                                in_=C1[0:64, s0:s0 + ncopy])

        # ---- conv3 + softmax: K = exp(l-4) / sum ----
        # split in two phases so the PE never waits on the E activation:
        # logits+exp are emitted first, conv blocks fill the PE while the
        # ACT runs, then the sum-matmul finds E already in SBUF.
        def c3_logits_exp(r, nr):
            N = nr * Wp
            co = 1 + r * Wp
            lg = psum3.tile([9, 512], F32, tag="c3a")
            nc.tensor.matmul(lg[:, :N], W3[:, :], C2[:, co:co + N],
                             start=True, stop=True)
            E = mid.tile([9, 512], DT16, tag="E")
            nc.scalar.activation(E[:, :N], lg[:, :N], AF.Exp, bias=b3)
            return E

        def c3_sum_div(r, nr, E, KS, kcol):
            N = nr * Wp
            sp9 = psums.tile([9, 512], F32, tag="s")
            nc.tensor.matmul(sp9[:, :N], ones9, E[:, :N], start=True, stop=True)
            # DVE has no divide ALU: native reciprocal (straight from PSUM,
            # replicated to all 9 partitions by the ones-matmul) + multiply
            R9 = mid.tile([9, 512], DT16, tag="R")
            with nc.allow_low_precision(reason="1/S in fp16: S in [1e-2, 5e2]"):
                nc.vector.reciprocal(R9[:, :N], sp9[:, :N])
            nc.vector.tensor_mul(KS[:, kcol:kcol + N], E[:, :N], R9[:, :N])

        # ---- tap reduce: A[c,f] = sum_t K[t,f] * x[c, f+delta_t] ----
        def bcast_block(r0, nr):
            F = nr * Wp
            KB = kbp.tile([C, 2 + 9 * RB * Wp], DT16, tag="kb")
            src = kdram[0:9, r0 * Wp:r0 * Wp + F].partition_broadcast(C)
            nc.gpsimd.dma_start(
                out=KB[:, 1:1 + 9 * F].rearrange("p (t f) -> p t f", f=F), in_=src)
            return KB

        def reduce_block(r0, nr, KB):
            # 7 DVE ops instead of 17: each op handles 3 tap-planes at once
            # (x taps via an overlapping stride-1 tap dim; products written
            # in-place over KB's k planes), then a 2-level fold into A.
            F = nr * Wp
            xps = list(XB.ap)[0][0]
            kps = list(KB.ap)[0][0]

            def x3(di):
                return bass.AP(XB.tensor, XB.offset + 1 + (r0 + di) * Wp,
                               [[xps, C], [1, 3], [Wp, nr], [1, W]])

            def k3(di):
                return bass.AP(KB.tensor, KB.offset + 1 + (3 * di) * F + 1,
                               [[kps, C], [F, 3], [Wp, nr], [1, W]])

            def k1(t):
                return bass.AP(KB.tensor, KB.offset + 1 + t * F + 1,
                               [[kps, C], [Wp, nr], [1, W]])

            for di in range(3):
                nc.vector.tensor_mul(k3(di), x3(di), k3(di))
            nc.vector.tensor_add(k3(0), k3(0), k3(1))
            nc.vector.tensor_add(k3(0), k3(0), k3(2))
            av = rview(A, 1 + (r0 + 1) * Wp + 1, nr, 0, W)
            nc.vector.tensor_add(av, k1(0), k1(1))
            nc.vector.tensor_add(av, av, k1(2))

        def f2_block(r, nr):
            PT = psum.tile([128, 1024], F32, tag="mm")
            halves = []
            rh, left, base = r, nr, 0
            while left > 0:
                nh = min(2, left)
                N = nh * Wp
                for t in range(9):
                    di, dj = divmod(t, 3)
                    off = 1 + (rh + di) * Wp + (dj - 1)
                    nc.tensor.matmul(PT[:, base:base + N], F2W[:, t * 128:(t + 1) * 128],
                                     F1[:, off:off + N], start=(t == 0), stop=(t == 8))
                halves.append((base, nh))
                rh += nh
                left -= nh
                base += 512
            ST = stg.tile([128, 4 * W], F32, tag="st")
            if len(halves) == 2 and halves[1][1] == 2:
                pin = PT[:, 0:1024].rearrange("p (b n) -> p b n", n=512)[:, :, 0:2 * Wp] \
                    .rearrange("p b (r w) -> p b r w", w=Wp)[:, :, :, 1:1 + W]
                pout = ST[:, 0:4 * W].rearrange("p (b r w) -> p b r w", b=2, w=W)
                nc.scalar.activation(pout, pin, AF.Identity, bias=fb2)
            else:
                row = 0
                for base, nh in halves:
                    N = nh * Wp
                    psrc = PT[:, base:base + N].rearrange(
                        "p (r w) -> p r w", w=Wp)[:, :, 1:1 + W]
                    pdst = ST[:, row * W:(row + nh) * W].rearrange(
                        "p (r w) -> p r w", w=W)
                    nc.scalar.activation(pdst, psrc, AF.Identity, bias=fb2)
                    row += nh
            nc.sync.dma_start(out=out_ext[:, r * W:(r + nr) * W], in_=ST[:, :nr * W])

        # f1 starts with a 2-row block so it can begin right after the first
        # (also 2-row) reduce chunk instead of waiting for 5 rows of A
        blocksf1 = [(0, 2)] + [(r, min(4, rows_f1 - r)) for r in range(2, rows_f1, 4)]
        # the very last f2 block runs as two 2-row pieces so the closing
        # ACT->out-DMA->drain chain is half as long
        blocksf2 = [(r, 4) for r in range(0, rows_f2 - 4, 4)] \
            + [(rows_f2 - 4, 2), (rows_f2 - 2, 2)]
        state = {"if1": 0, "if2": 0, "f1_rows": 0}

        def emit_one_f(a_rows, final=False):
            if state["if2"] < len(blocksf2):
                r2, n2 = blocksf2[state["if2"]]
                if final or r2 + n2 <= state["f1_rows"] - 1:
                    f2_block(r2, n2)
                    state["if2"] += 1
                    return True
            if state["if1"] < len(blocksf1):
                r, nr = blocksf1[state["if1"]]
                if final or r + nr <= a_rows - 1:
                    conv_block4(r, nr, F1W, 128, 128, A, fb1, F1, AF.Relu)
                    state["if1"] += 1
                    state["f1_rows"] = r + nr
                    return True
            return False

        # ---- fully pipelined emission ----
        n_c1 = (rows_c1 + 3) // 4      # 25
        n_c2 = (rows_c2 + 3) // 4      # 25
        i_c1 = i_c2 = 0
        sub_r = 0

        def c2_step(j):
            r = 4 * j
            c2_block4(r, min(4, rows_c2 - r))

        # prologue below builds a 6-block c1 / 4-block c2 lead, with the
        # first K-chain launched as early as its c2 dependency allows so its
        # logits matmul doesn't queue behind the whole prologue

        # first 4 k-rows as 2-row chunks (earlier first A rows), then 4-row
        sblocks = [(0, 2), (2, 2)]
        sblocks += [(r0, min(RB, rows_k - r0)) for r0 in range(4, rows_k, RB)]
        n_sb = len(sblocks)
        cstate = {"c1": i_c1, "c2": i_c2}
        kb_q = []

        def filler():
            if cstate["c1"] < n_c1:
                c1_block4(cstate["c1"]); cstate["c1"] += 1
            if cstate["c2"] < n_c2:
                c2_step(cstate["c2"]); cstate["c2"] += 1

        def k_front(ik, pe_filler=None):
            # c3 chain + kdram + broadcast for superblock ik, emitted one
            # superblock ahead of its reduce so the DVE divides precede the
            # previous reduce's TT batch and the broadcast overlaps it
            r0, nr = sblocks[ik]
            KS = ksp.tile([9, RB * Wp + 2], DT16, tag="ks")
            subs = []
            rr = r0
            while rr < r0 + nr:
                n2 = min(2, rows_k - rr)
                subs.append((rr, n2, c3_logits_exp(rr, n2)))
                rr += n2
            if pe_filler:
                pe_filler()          # PE work while the exp activations drain
            for (rr, n2, E) in subs:
                c3_sum_div(rr, n2, E, KS, (rr - r0) * Wp)
            nc.sync.dma_start(out=kdram[0:9, r0 * Wp:r0 * Wp + nr * Wp],
                              in_=KS[0:9, 0:nr * Wp])
            kb_q.append((r0, nr, bcast_block(r0, nr)))

        for _ in range(5):
            c1_block4(cstate["c1"]); cstate["c1"] += 1
        for _ in range(2):
            c2_step(cstate["c2"]); cstate["c2"] += 1
        k_front(0)
        k_front(1, filler)
        k_front(2, filler)
        k_rows = sblocks[0][1] + sblocks[1][1] + sblocks[2][1]
        for s in range(n_sb):
            # bulk chunks BEFORE this iter's k_front so every XB row is
            # emitted ahead of its first reader (the filler's c1 block);
            # SP.SEQ order still keeps them behind the PREVIOUS kdram write
            emit_xchunk(8)
            emit_xchunk(8)
            if s + 3 < n_sb:
                k_front(s + 3, filler)
                k_rows += sblocks[s + 3][1]
            else:
                filler()
            if s == 0:
                nc.sync.dma_start(out=F1W[:], in_=f1_ext[:])
            elif s == 1:
                nc.sync.dma_start(out=F2W[:], in_=f2_ext[:])
            reduce_block(*kb_q.pop(0))
            # f blocks trail the reduce frontier by 14 rows (~3.5 superblocks)
            # so the in-order PE never parks on an f block whose reduce is
            # still in DVE's queue
            a_gate = k_rows - sblocks[s][1] - 14
            emit_one_f(a_gate)
            emit_one_f(a_gate)
        # drain: emit_one_f's f2-first gating yields the stall-minimal
        # interleaving of the remaining blocks
        while emit_one_f(rows_k):
            pass
        while state["if1"] < len(blocksf1) or state["if2"] < len(blocksf2):
            emit_one_f(rows_k, final=True)

    if compile_:
        nc.compile()
    return nc


# ==========================================================================
# host side
# ==========================================================================
import numpy as np
from concourse.bass_utils import run_bass_kernel_spmd

NP16 = np.float16
B, C, H, W = 4, 128, 192, 192
HB = H // 2
RIN = HB + 4
Wp = W + 2

_FLIP_PERM = np.array([3 * (2 - (t // 3)) + (t % 3) for t in range(9)])

LOGIT_SHIFT = 4.0   # exp(l - 4): safe for logits in (-12.5, 12.9) per-pixel max


def _flip_taps(w):
    return np.ascontiguousarray(w[:, :, ::-1, :])


def _pack_weights(w1, b1, w2, b2, w3, b3, f1, fb1, f2, fb2, flipped):
    if flipped:
        w1, w2, f1, f2 = _flip_taps(w1), _flip_taps(w2), _flip_taps(f1), _flip_taps(f2)
        w3 = w3[_FLIP_PERM]
        b3 = b3[_FLIP_PERM]
    d = {}
    d["w1l"] = np.ascontiguousarray(
        w1.transpose(1, 2, 3, 0).reshape(C, 9 * 64)).astype(NP16)
    # conv2 K-packed lhsT [128, 6*32]: cols dj*32 hold the di=0/1 pair
    # (rows 0-63 di=0, rows 64-127 di=1); cols (3+dj)*32 hold di=2 (rows 0-63)
    w2p = np.zeros((128, 6 * 32), np.float32)
    for dj in range(3):
        w2p[0:64, dj * 32:(dj + 1) * 32] = w2[:, :, 0, dj].T
        w2p[64:128, dj * 32:(dj + 1) * 32] = w2[:, :, 1, dj].T
        w2p[0:64, (3 + dj) * 32:(4 + dj) * 32] = w2[:, :, 2, dj].T
    d["w2l"] = w2p.astype(NP16)
    d["w3l"] = np.ascontiguousarray(w3[:, :, 0, 0].T).astype(NP16)
    d["f1l"] = np.ascontiguousarray(
        f1.transpose(1, 2, 3, 0).reshape(C, 9 * C)).astype(NP16)
    d["f2l"] = np.ascontiguousarray(
        f2.transpose(1, 2, 3, 0).reshape(C, 9 * C)).astype(NP16)
    ones = np.zeros((16, 16), np.float32)
    ones[0:9, 0:9] = 1.0
    d["ones"] = ones.astype(NP16)
    bias = np.zeros((C, 8), np.float32)
    bias[0:64, 0] = b1
    bias[0:32, 1] = b2
    # logit shift keeps exp() within fp16 range; cancels exactly in K = E/S
    bias[0:9, 2] = b3 - LOGIT_SHIFT
    bias[0:C, 3] = fb1
    bias[0:C, 4] = fb2
    d["bias"] = bias
    return d


def _pack_x(x_loc):
    nX = RIN + 1
    xb = np.zeros((C, 1 + nX * Wp + 1), np.float32)
    v = xb[:, 1:1 + nX * Wp].reshape(C, nX, Wp)
    v[:, 1:, 1:1 + W] = x_loc
    return xb.astype(NP16)


_NC_CACHE = {}


def _get_nc():
    if "nc" not in _NC_CACHE:
        _NC_CACHE["nc"] = build_kernel()
    return _NC_CACHE["nc"]


def _make_in_maps(x, w1, b1, w2, b2, w3, b3, f1, fb1, f2, fb2):
    x = np.asarray(x, np.float32)
    args = [np.asarray(a, np.float32) for a in
            (w1, b1, w2, b2, w3, b3, f1, fb1, f2, fb2)]
    wtop = _pack_weights(*args, flipped=False)
    wbot = _pack_weights(*args, flipped=True)
    in_maps = []
    for b in range(B):
        for j in range(2):
            if j == 0:
                xloc = x[b, :, 0:RIN, :]
                wd = wtop
            else:
                xloc = np.ascontiguousarray(x[b, :, H - RIN:H, :][:, ::-1, :])
                wd = wbot
            m = dict(wd)
            m["xb"] = _pack_x(xloc)
            in_maps.append(m)
    return in_maps


def _assemble(results):
    out = np.zeros((B, C, H, W), np.float32)
    for b in range(B):
        out[b, :, 0:HB, :] = results[2 * b]["out"].reshape(C, HB, W)
        out[b, :, HB:H, :] = results[2 * b + 1]["out"].reshape(C, HB, W)[:, ::-1, :]
    return out


def kernel(x, w1, b1, w2, b2, w3, b3, f1, fb1, f2, fb2):
    nc = _get_nc()
    in_maps = _make_in_maps(x, w1, b1, w2, b2, w3, b3, f1, fb1, f2, fb2)
    res = run_bass_kernel_spmd(nc, in_maps, core_ids=list(range(8)))
    return _assemble(res.results)
